# revision 1
# baseline (speedup 1.0000x reference)
"""Trainium2 Bass kernel for CRPExpertAggregator (moe_routing).

Full-input contract: kernel(**inputs) takes the full unsharded inputs and
returns the full (256, 100) logits. Internally shards batch 8 ways across
NeuronCores 0-7 (data parallel; expert params replicated) and runs one SPMD
Bass program via concourse.bass_utils.run_bass_kernel_spmd.

Math (identical to the reference up to fp reassociation):
  H = x.reshape(B, 64, 256)
  scores[b,el,s] = sum_a (q@Wk)[el,a] * H[b,s,a] / 16      (K never formed)
  attn = softmax_s(scores);  attn_avg[bs,e] = 0.25*sum_l attn
  U[b,e,a] = sum_s attn_avg * H;  z[b,e,d] = sum_a U * WvT  (V never formed)
  raw = ||z||, allsc = raw * log(counts+2), top-3 gate, logits = final @ cqT

Precision: the scores path runs in fp16 (softmax of tiny scores is insensitive),
the U/z/raw path runs in fp32 — the top-3 gate margins are as small as 6e-4 on
scores of magnitude ~6, and any fp16 rounding there flips expert selection
vs the fp32 reference.
"""

import os
import numpy as np

import concourse.bass as bass
import concourse.bacc as bacc
import concourse.mybir as mybir
import concourse.tile as tile
from concourse.bass_utils import run_bass_kernel_spmd
from concourse.alu_op_type import AluOpType

FP32 = mybir.dt.float32
FP16 = mybir.dt.float16
I32 = mybir.dt.int32
AF = mybir.ActivationFunctionType
AX = mybir.AxisListType

N_CORES = 8
B = 256            # full batch
BL = B // N_CORES  # 32 rows per core
S = 64             # slots
A = 256            # agent dim (contraction for projections)
D = 256            # embed dim
E = 16             # experts
L = 4              # queries per expert
C = 100            # classes
R = BL * S         # 2048 H-rows per core
P = 128
SCALE = 1.0 / 16.0  # 1/sqrt(D)


def _build_program():
    nc = bacc.Bacc("TRN2", debug=False, enable_asserts=False, num_devices=N_CORES)

    xT = nc.dram_tensor("xT", (A, R), FP16, kind="ExternalInput").ap()
    xn = nc.dram_tensor("xn", (R, A), FP32, kind="ExternalInput").ap()
    wk = nc.dram_tensor("wk", (E, D, A), FP16, kind="ExternalInput").ap()
    wvT = nc.dram_tensor("wvT", (E, A, D), FP32, kind="ExternalInput").ap()
    qT = nc.dram_tensor("qT", (E, D, L), FP16, kind="ExternalInput").ap()
    cqT = nc.dram_tensor("cqT", (D, C), FP32, kind="ExternalInput").ap()
    cnt = nc.dram_tensor("cnt", (BL, E), I32, kind="ExternalInput").ap()
    selp = nc.dram_tensor("selp", (P, E), FP32, kind="ExternalInput").ap()
    s4 = nc.dram_tensor("s4", (P, BL), FP32, kind="ExternalInput").ap()
    out = nc.dram_tensor("out", (BL, C), FP32, kind="ExternalOutput").ap()

    with tile.TileContext(nc) as tc:
        with tc.tile_pool(name="sb", bufs=1) as sb, \
             tc.tile_pool(name="ps", bufs=1, space="PSUM") as ps:
            # ---------------- DMA inputs ----------------
            qt_sb = sb.tile([P, 2, E, L], FP16)
            qt_r = qT.rearrange("e (dc dp) l -> dp dc e l", dp=P)
            for dc in range(2):
                nc.sync.dma_start(qt_sb[:, dc], qt_r[:, dc])
            selp_sb = sb.tile([P, E], FP32)
            nc.sync.dma_start(selp_sb, selp)
            s4_sb = sb.tile([P, BL], FP32)
            nc.sync.dma_start(s4_sb, s4)
            cqt_sb = sb.tile([P, 2, C], FP32)
            nc.sync.dma_start(cqt_sb, cqT.rearrange("(dc dp) c -> dp dc c", dp=P))
            cnt_sb = sb.tile([BL, E], I32)
            nc.sync.dma_start(cnt_sb, cnt)

            # per-expert Wk so QW matmuls can start while later experts stream
            wk_sb = sb.tile([P, 2, E, A], FP16)  # [d_p, d_c, e, a]
            wk_r = wk.rearrange("e (dc dp) a -> dp dc e a", dp=P)
            for e in range(E):
                nc.sync.dma_start(wk_sb[:, :, e], wk_r[:, :, e])

            ht_sb = sb.tile([P, 2, R], FP16)  # H^T: [a_p, a_c, bs]
            ht_r = xT.rearrange("(ac ap) r -> ap ac r", ap=P)
            for ac in range(2):
                nc.sync.dma_start(ht_sb[:, ac], ht_r[:, ac])

            h_sb = sb.tile([P, R // P, A], FP32)  # H natural: [bs_p, bs_c, a]
            h_r = xn.rearrange("(rc rp) a -> rp rc a", rp=P)
            for rc2 in range(4):
                nc.sync.dma_start(h_sb[:, 4 * rc2:4 * (rc2 + 1)],
                                  h_r[:, 4 * rc2:4 * (rc2 + 1)])

            wv_sb = sb.tile([P, 2, E, D], FP32)  # Wv^T: [a_p, a_c, e, d]
            wv_r = wvT.rearrange("e (ac ap) d -> ap ac e d", ap=P)
            for e in range(E):
                nc.sync.dma_start(wv_sb[:, :, e], wv_r[:, :, e])

            # ---------------- QW^T = (q @ Wk)^T / 16 : [a, e, l] (fp16) --------
            qwt_sb = sb.tile([P, 2, E, L], FP16)
            for ac in range(2):
                pq = ps.tile([P, E, L], FP32, tag="gp", bufs=3)
                for e in range(E):
                    for dc in range(2):
                        nc.tensor.matmul(
                            pq[:, e, :],
                            wk_sb[:, dc, e, ac * P:(ac + 1) * P],
                            qt_sb[:, dc, e, :],
                            start=(dc == 0), stop=(dc == 1),
                        )
                nc.vector.tensor_scalar_mul(qwt_sb[:, ac], pq, SCALE)

            # ---------------- scores (fp16 mm) -> exp -> normalize (fp32) ------
            # attn layout [el=64 (rows 64:128 zero), b=32, s=64]
            attn_sb = sb.tile([P, BL, S], FP32)
            nc.vector.memset(attn_sb[S:P], 0.0)
            den = sb.tile([S, BL], FP32)
            rden = sb.tile([S, BL], FP32)
            for rc in range(4):  # 512-wide bs chunks
                psc = ps.tile([S, 8, S], FP32, tag="sc", bufs=2)
                for ac in range(2):
                    nc.tensor.matmul(
                        psc.rearrange("p b s -> p (b s)"),
                        qwt_sb[:, ac].rearrange("p e l -> p (e l)"),
                        ht_sb[:, ac, 512 * rc:512 * (rc + 1)],
                        start=(ac == 0), stop=(ac == 1),
                    )
                bs_sl = slice(8 * rc, 8 * (rc + 1))
                nc.scalar.activation(attn_sb[:S, bs_sl], psc, AF.Exp)
                nc.vector.reduce_sum(den[:, bs_sl], attn_sb[:S, bs_sl], axis=AX.X)
                nc.vector.reciprocal(rden[:, bs_sl], den[:, bs_sl])
                nc.vector.tensor_tensor(
                    attn_sb[:S, bs_sl], attn_sb[:S, bs_sl],
                    rden[:, bs_sl, None].to_broadcast((S, 8, S)),
                    AluOpType.mult,
                )

            # ------- attn_avg^T [bs, e] = 0.25 * sum_l attn, parity-masked -----
            # avt_both[p, rc, par, e]: par=0 valid on rows 0:64 (b even), par=1
            # on rows 64:128 (b odd); the complementary rows are zero so the
            # U matmul can contract over all 128 partitions.
            avt_both = sb.tile([P, R // P, 2, E], FP32)
            nc.vector.memset(avt_both[S:P, :, 0, :], 0.0)
            nc.vector.memset(avt_both[:S, :, 1, :], 0.0)
            for half in range(2):
                pav = ps.tile([P, 8, E], FP32, tag="gp", bufs=3)
                for i in range(8):
                    rc8 = 8 * half + i
                    nc.tensor.matmul(
                        pav[:, i, :],
                        attn_sb[:, 2 * rc8:2 * rc8 + 2, :]
                        .rearrange("p b s -> p (b s)"),
                        selp_sb,
                        start=True, stop=True,
                    )
                h_sl = slice(8 * half, 8 * (half + 1))
                nc.vector.tensor_copy(avt_both[:S, h_sl, 0, :], pav[:S])
                nc.vector.tensor_copy(avt_both[S:P, h_sl, 1, :], pav[S:P])

            # ---------------- U^T [a, b, e] = sum_s H^T attn_avg (fp32) --------
            ut_sb = sb.tile([P, 2, E, BL], FP32)  # [a_p, a_c, e, b]
            for ac in range(2):
                for half in range(2):
                    pu = ps.tile([P, 8, 2, E], FP32, tag="gp", bufs=3)
                    for i in range(8):
                        rc = 8 * half + i
                        nc.tensor.matmul(
                            pu[:, i, :, :].rearrange("p par e -> p (par e)"),
                            h_sb[:, rc, ac * P:(ac + 1) * P],
                            avt_both[:, rc, :, :].rearrange("p par e -> p (par e)"),
                            start=True, stop=True,
                        )
                    # pu[p, rc8, par, e] -> ut[p, ac, e, b=2*rc+par]
                    nc.vector.tensor_copy(
                        ut_sb[:, ac, :, 16 * half:16 * (half + 1)]
                        .rearrange("p e (rc par) -> p rc par e", par=2),
                        pu)

            # ------------- z [32j+b, t, d], expert e = 4t+j (fp32) -------------
            z_sb = sb.tile([P, 4, D], FP32)
            for t in range(4):
                pz = ps.tile([P, D], FP32, tag="z", bufs=3)
                for j in range(4):
                    e = 4 * t + j
                    for ac in range(2):
                        nc.tensor.matmul(
                            pz[32 * j:32 * (j + 1), :],
                            ut_sb[:, ac, e, :],
                            wv_sb[:, ac, e, :],
                            start=(ac == 0), stop=(ac == 1),
                            tile_position=(0, 32 * j),
                        )
                nc.vector.tensor_copy(z_sb[:, t, :], pz)

            # ---------------- raw = ||z||, allsc = raw * log(cnt+2) ------------
            zsq = sb.tile([P, 4, D], FP32)
            nc.scalar.activation(zsq, z_sb, AF.Square)
            rawsq = sb.tile([P, 4], FP32)
            nc.vector.reduce_sum(rawsq, zsq, axis=AX.X)
            raw = sb.tile([P, 4], FP32)
            nc.scalar.sqrt(raw, rawsq)

            cntf = sb.tile([BL, E], FP32)
            nc.vector.tensor_copy(cntf, cnt_sb)
            cnt2 = sb.tile([BL, E], FP32)
            nc.vector.tensor_scalar_add(cnt2, cntf, 2.0)
            crp32 = sb.tile([BL, E], FP32)
            nc.scalar.activation(crp32, cnt2, AF.Ln)

            rw2 = sb.tile([BL, 4, 4], FP32)  # [b, t, j] -> free index e=4t+j
            for j in range(4):
                nc.vector.tensor_copy(rw2[:, :, j], raw[32 * j:32 * (j + 1), :])
            allsc = sb.tile([BL, E], FP32)
            nc.vector.tensor_tensor(
                allsc.rearrange("p (t j) -> p t j", j=4), rw2,
                crp32.rearrange("p (t j) -> p t j", j=4), AluOpType.mult)

            # ---------------- top-3 gate ----------------
            mx8 = sb.tile([BL, 8], FP32)
            nc.vector.max(mx8, allsc)
            negm1 = sb.tile([BL, 1], FP32)
            nc.vector.tensor_scalar_mul(negm1, mx8[:, 0:1], -1.0)
            g = sb.tile([BL, E], FP32)
            nc.scalar.activation(g, allsc, AF.Exp, bias=negm1)
            mask = sb.tile([BL, E], FP32)
            nc.vector.tensor_scalar(mask, allsc, mx8[:, 2:3], None, AluOpType.is_ge)
            gm = sb.tile([BL, E], FP32)
            nc.vector.tensor_mul(gm, g, mask)
            ssum = sb.tile([BL, 1], FP32)
            nc.vector.reduce_sum(ssum, gm, axis=AX.X)
            rsum = sb.tile([BL, 1], FP32)
            nc.vector.reciprocal(rsum, ssum)
            we = sb.tile([BL, E], FP32)
            nc.vector.tensor_scalar_mul(we, gm, rsum)

            # scatter we [b, e] -> we128 [32j+b, t]
            we128 = sb.tile([P, 4], FP32)
            wev = we.rearrange("p (t j) -> p t j", j=4)
            for j in range(4):
                nc.vector.tensor_copy(we128[32 * j:32 * (j + 1), :], wev[:, :, j])
            # weighted selector wsel[p, t, b] = s4[p, b] * we128[p, t]
            wsel = sb.tile([P, 4, BL], FP32)
            for t in range(4):
                nc.vector.tensor_scalar_mul(wsel[:, t, :], s4_sb, we128[:, t:t + 1])

            # final^T [d, b] = sum_{p,t} z[p, t, d] * wsel[p, t, b]
            pft = ps.tile([P, 2, BL], FP32, tag="gp", bufs=3)
            for dc in range(2):
                for t in range(4):
                    nc.tensor.matmul(
                        pft[:, dc, :],
                        z_sb[:, t, dc * P:(dc + 1) * P],
                        wsel[:, t, :],
                        start=(t == 0), stop=(t == 3),
                    )
            ft_sb = sb.tile([P, 2, BL], FP32)
            nc.vector.tensor_copy(ft_sb, pft)

            # logits [b, c] = sum_d final^T[d, b] * cq^T[d, c]
            plog = ps.tile([BL, C], FP32, tag="gp", bufs=3)
            for dc in range(2):
                nc.tensor.matmul(
                    plog, ft_sb[:, dc, :], cqt_sb[:, dc, :],
                    start=(dc == 0), stop=(dc == 1),
                )
            out_sb = sb.tile([BL, C], FP32)
            nc.vector.tensor_copy(out_sb, plog)
            nc.sync.dma_start(out, out_sb)

    nc.compile()
    # compile()'s move_matmul_waits_to_ldweights runs before the final ISA
    # lowering splits fused matmuls into Ldweights+Matmult, so a matmul can
    # still carry 2 waits (walrus MM struct fits only 1). Re-run the passes.
    import bass_rust
    bass_rust.move_matmul_waits_to_ldweights(nc.m)
    bass_rust.generate_event_semaphores(nc)
    for f in nc.m.functions:
        for blk in f.blocks:
            for inst in blk.instructions:
                w = inst.sync_info.on_wait if inst.sync_info else None
                if w and len(w) > 1 and "EventSemaphore" not in str(inst.opcode):
                    raise RuntimeError(
                        f"{inst.name} {inst.opcode} still has {len(w)} waits")
    return nc


_NC = None


def _get_nc():
    global _NC
    if _NC is None:
        _NC = _build_program()
    return _NC


def _host_consts():
    sel = np.zeros((P, E), np.float32)
    for el in range(S):
        sel[el, el // L] = 1.0 / L
    s4 = np.tile(np.eye(BL, dtype=np.float32), (4, 1))
    return sel, s4


def _make_in_maps(inputs):
    x = np.ascontiguousarray(np.asarray(inputs["x"], dtype=np.float32))
    queries = np.asarray(inputs["queries"], dtype=np.float32)
    Wk = np.ascontiguousarray(np.asarray(inputs["Wk"], dtype=np.float32))
    Wv = np.asarray(inputs["Wv"], dtype=np.float32)
    cq = np.asarray(inputs["class_queries"], dtype=np.float32)
    counts = np.ascontiguousarray(np.broadcast_to(
        np.asarray(inputs["expert_counts"]).astype(np.int32).reshape(1, E), (BL, E)))

    qT = np.ascontiguousarray(queries.transpose(0, 2, 1)).astype(np.float16)
    wvT = np.ascontiguousarray(Wv.transpose(0, 2, 1))            # fp32
    wk16 = Wk.astype(np.float16)
    cqT = np.ascontiguousarray(cq.T)                             # fp32
    sel, s4 = _host_consts()

    in_maps = []
    for c in range(N_CORES):
        xl = x[BL * c:BL * (c + 1)].reshape(R, A)
        in_maps.append({
            "xT": np.ascontiguousarray(xl.T).astype(np.float16),
            "xn": xl,
            "wk": wk16,
            "wvT": wvT,
            "qT": qT,
            "cqT": cqT,
            "cnt": counts,
            "selp": sel,
            "s4": s4,
        })
    return in_maps


def run_sharded(inputs, trace=False, **kwargs):
    nc = _get_nc()
    in_maps = _make_in_maps(inputs)
    res = run_bass_kernel_spmd(nc, in_maps, core_ids=list(range(N_CORES)),
                               trace=trace, **kwargs)
    outs = np.concatenate([res.results[c]["out"] for c in range(N_CORES)], axis=0)
    return outs.astype(np.float32), res


def kernel(**inputs):
    out, _ = run_sharded(inputs, trace=False)
    return out



# revision 3
# speedup vs baseline: 1.3727x; 1.3727x over previous
"""Trainium2 Bass kernel for CRPExpertAggregator (moe_routing).

Full-input contract: kernel(**inputs) takes the full unsharded inputs and
returns the full (256, 100) logits. Internally shards batch 8 ways across
NeuronCores 0-7 (data parallel; expert params replicated) and runs one SPMD
Bass program via concourse.bass_utils.run_bass_kernel_spmd.

Math (identical to the reference up to fp reassociation):
  H = x.reshape(B, 64, 256)
  scores[b,el,s] = sum_a (q@Wk/16)[el,a] * H[b,s,a]         (K never formed)
  attn = softmax_s(scores);  attn_avg[bs,e] = 0.25*sum_l attn
  U[b,e,a] = sum_s attn_avg * H;  z[b,e,d] = sum_a U * WvT  (V never formed)
  raw = ||z||, allsc = raw * log(counts+2), top-3 gate, logits = final @ cqT

Precision: the scores path streams fp16 (per-row margin analysis gives a
137x safety factor on the top-3 gate); the U/z path streams fp32 — rounding
xn or Wv to fp16 leaves < 1.6x margin against the smallest rank-3/4 gap and
would risk flipping expert selection vs the fp32 reference.

Perf structure (vs the 77.6us predecessor):
  - q@Wk and log(counts+2) precomputed on host (kills 64 matmuls + 2MB DMA)
  - all DRAM buffers pre-packed host-side into exact SBUF layout: each
    dma_start is 128 contiguous 4-8KB descriptors instead of thousands of
    0.5-1KB rows (the old kernel moved 9.2MB in 15.9k descriptors)
  - scores matmuls 2-way column-tiled (m=64), attn-average 2-way row-tiled
    (k=64), z matmuls 4-way column-tiled (m=32): independent PE tiles
    execute concurrently
  - Wv DMA ordered by expert group so z t-group t starts as soon as its 4
    experts' weights land
  - Exp/Sqrt activation tables prefetched with dummy activations so table
    loads hide under DMA / the z section
"""

import numpy as np

import concourse.bass as bass
import concourse.bacc as bacc
import concourse.mybir as mybir
import concourse.tile as tile
from concourse.bass_utils import run_bass_kernel_spmd
from concourse.alu_op_type import AluOpType

FP32 = mybir.dt.float32
FP16 = mybir.dt.float16
AF = mybir.ActivationFunctionType
AX = mybir.AxisListType

N_CORES = 8
B = 256            # full batch
BL = B // N_CORES  # 32 rows per core
S = 64             # slots
A = 256            # agent dim (contraction for projections)
D = 256            # embed dim
E = 16             # experts
L = 4              # queries per expert
C = 100            # classes
R = BL * S         # 2048 H-rows per core
P = 128


def _build_program():
    nc = bacc.Bacc("TRN2", debug=False, enable_asserts=False, num_devices=N_CORES)

    # Host-packed DRAM inputs (exact SBUF layouts, partition dim first).
    xT = nc.dram_tensor("xT", (P, 2, R), FP16, kind="ExternalInput").ap()
    xn = nc.dram_tensor("xn", (P, R // P, A), FP32, kind="ExternalInput").ap()
    wv = nc.dram_tensor("wv", (P, E, 2, D), FP32, kind="ExternalInput").ap()
    c16 = nc.dram_tensor("c16", (P, 2 * S + 2 * C), FP16, kind="ExternalInput").ap()
    c32 = nc.dram_tensor("c32", (P, 64), FP32, kind="ExternalInput").ap()
    out = nc.dram_tensor("out", (BL, C), FP32, kind="ExternalOutput").ap()

    with tile.TileContext(nc) as tc:
        with tc.tile_pool(name="sb", bufs=1) as sb, \
             tc.tile_pool(name="ps", bufs=1, space="PSUM") as ps:
            # ---------------- DMA inputs (order = priority) ----------------
            c16_sb = sb.tile([P, 2 * S + 2 * C], FP16)
            nc.sync.dma_start(c16_sb, c16)
            xt_sb = sb.tile([P, 2, R], FP16)
            for ac in range(2):
                nc.sync.dma_start(xt_sb[:, ac], xT[:, ac])
            c32_sb = sb.tile([P, 64], FP32)
            nc.sync.dma_start(c32_sb, c32)
            xn_sb = sb.tile([P, R // P, A], FP32)
            for h in range(2):
                nc.sync.dma_start(xn_sb[:, 8 * h:8 * (h + 1)], xn[:, 8 * h:8 * (h + 1)])
            wv_sb = sb.tile([P, E, 2, D], FP32)
            for g in range(4):
                nc.sync.dma_start(wv_sb[:, 4 * g:4 * (g + 1)], wv[:, 4 * g:4 * (g + 1)])

            qwt = c16_sb[:, 0:2 * S].rearrange("p (ac el) -> p ac el", ac=2)
            cqt = c16_sb[:, 2 * S:].rearrange("p (dc c) -> p dc c", dc=2)
            selp = c32_sb[:, 0:E]
            s4 = c32_sb[:, E:E + BL]
            crp = c32_sb[0:BL, 48:48 + E]

            # Prefetch the Exp activation table while DMAs stream.
            warm_in = sb.tile([1, 1], FP32)
            warm_out = sb.tile([1, 2], FP32)
            nc.vector.memset(warm_in, 0.0)
            nc.scalar.activation(warm_out[:, 0:1], warm_in, AF.Exp)

            # ------- scores (fp16 mm, 2-way col tiling) -> exp -> normalize ----
            # attn layout [p = 64*h + el, bb = b%16, s]; h = b//16.
            attn = sb.tile([P, E, S], FP32)
            den = sb.tile([P, E], FP32)
            rden = sb.tile([P, E], FP32)
            for it in range(2):
                psc = ps.tile([P, 8, S], FP32, tag="sc", bufs=2)
                for h in range(2):
                    for ac in range(2):
                        nc.tensor.matmul(
                            psc[64 * h:64 * (h + 1)].rearrange("p b s -> p (b s)"),
                            qwt[:, ac, :],
                            xt_sb[:, ac, 1024 * h + 512 * it:1024 * h + 512 * (it + 1)],
                            start=(ac == 0), stop=(ac == 1),
                            tile_position=(0, 64 * h),
                        )
                sl = slice(8 * it, 8 * (it + 1))
                nc.scalar.activation(attn[:, sl, :], psc, AF.Exp)
                nc.vector.reduce_sum(den[:, sl], attn[:, sl, :], axis=AX.X)
                nc.vector.reciprocal(rden[:, sl], den[:, sl])
                nc.vector.tensor_tensor(
                    attn[:, sl, :], attn[:, sl, :],
                    rden[:, sl, None].to_broadcast((P, 8, S)),
                    AluOpType.mult,
                )

            # ------- attn_avg^T [bs, par, e] (2-way row tiling), parity-masked --
            # avt[r, rc, par, e] = attn_avg[b=2rc+par, s=r-64par, e]; the
            # complementary parity rows are zero so U can contract over all 128.
            avt = sb.tile([P, R // P, 2, E], FP32)
            nc.vector.memset(avt[S:P, :, 0, :], 0.0)
            nc.vector.memset(avt[:S, :, 1, :], 0.0)
            pav0 = ps.tile([P, 8, E], FP32, tag="gp", bufs=3)
            pav1 = ps.tile([P, 8, E], FP32, tag="gp", bufs=3)
            pav = [pav0, pav1]
            for it in range(2):
                for k in range(4):
                    pl = 4 * it + k
                    for h in range(2):
                        nc.tensor.matmul(
                            pav[h][:, pl, :],
                            attn[64 * h:64 * (h + 1), 2 * pl:2 * pl + 2, :]
                            .rearrange("p b s -> p (b s)"),
                            selp[64 * h:64 * (h + 1), :],
                            start=True, stop=True,
                            tile_position=(64 * h, 0),
                        )
            for h in range(2):
                hs = slice(8 * h, 8 * (h + 1))
                nc.vector.tensor_copy(avt[:S, hs, 0, :], pav[h][:S])
                nc.vector.tensor_copy(avt[S:P, hs, 1, :], pav[h][S:P])

            # ---------------- U^T [a, b, e] = sum_s H^T attn_avg (fp32) --------
            ut = sb.tile([P, 2, E, BL], FP32)  # [a_p, a_c, e, b]
            for ac in range(2):
                for half in range(2):
                    pu = ps.tile([P, 8, 2, E], FP32, tag="gp", bufs=3)
                    for i in range(8):
                        rc = 8 * half + i
                        nc.tensor.matmul(
                            pu[:, i, :, :].rearrange("p par e -> p (par e)"),
                            xn_sb[:, rc, 128 * ac:128 * (ac + 1)],
                            avt[:, rc, :, :].rearrange("p par e -> p (par e)"),
                            start=True, stop=True,
                        )
                    nc.vector.tensor_copy(
                        ut[:, ac, :, 16 * half:16 * (half + 1)]
                        .rearrange("p e (i par) -> p i par e", par=2),
                        pu)

            # Prefetch the Sqrt table while the z matmuls run.
            nc.scalar.sqrt(warm_out[:, 1:2], warm_in)

            # ------- z [32j+b, t, d], expert e = 4t+j (fp32, 4-way col tiling) -
            z_sb = sb.tile([P, 4, D], FP32)
            zsq = sb.tile([P, 4, D], FP32)
            rawsq = sb.tile([P, 4], FP32)
            for t in range(4):
                pz = ps.tile([P, D], FP32, tag="z", bufs=3)
                for j in range(4):
                    e = 4 * t + j
                    for ac in range(2):
                        nc.tensor.matmul(
                            pz[32 * j:32 * (j + 1), :],
                            ut[:, ac, e, :],
                            wv_sb[:, e, ac, :],
                            start=(ac == 0), stop=(ac == 1),
                            tile_position=(0, 32 * j),
                        )
                nc.scalar.activation(zsq[:, t, :], pz, AF.Square,
                                     accum_out=rawsq[:, t:t + 1])
                nc.vector.tensor_copy(z_sb[:, t, :], pz)

            # ---------------- raw = ||z||, allsc = raw * crp -------------------
            raw = sb.tile([P, 4], FP32)
            nc.scalar.sqrt(raw, rawsq)

            rw2 = sb.tile([BL, 4, 4], FP32)  # [b, t, j] -> free index e=4t+j
            for j in range(4):
                nc.vector.tensor_copy(rw2[:, :, j], raw[32 * j:32 * (j + 1), :])
            allsc = sb.tile([BL, E], FP32)
            nc.vector.tensor_tensor(
                allsc.rearrange("p (t j) -> p t j", j=4), rw2,
                crp.rearrange("p (t j) -> p t j", j=4), AluOpType.mult)

            # ---------------- top-3 gate ----------------
            mx8 = sb.tile([BL, 8], FP32)
            nc.vector.max(mx8, allsc)
            negm1 = sb.tile([BL, 1], FP32)
            nc.vector.tensor_scalar_mul(negm1, mx8[:, 0:1], -1.0)
            g = sb.tile([BL, E], FP32)
            nc.scalar.activation(g, allsc, AF.Exp, bias=negm1)
            mask = sb.tile([BL, E], FP32)
            nc.vector.tensor_scalar(mask, allsc, mx8[:, 2:3], None, AluOpType.is_ge)
            gm = sb.tile([BL, E], FP32)
            nc.vector.tensor_mul(gm, g, mask)
            ssum = sb.tile([BL, 1], FP32)
            nc.vector.reduce_sum(ssum, gm, axis=AX.X)
            rsum = sb.tile([BL, 1], FP32)
            nc.vector.reciprocal(rsum, ssum)
            we = sb.tile([BL, E], FP32)
            nc.vector.tensor_scalar_mul(we, gm, rsum)

            # scatter we [b, e] -> we128 [32j+b, t]; wsel[p, t, b] = s4 * we128
            we128 = sb.tile([P, 4], FP32)
            wev = we.rearrange("p (t j) -> p t j", j=4)
            for j in range(4):
                nc.vector.tensor_copy(we128[32 * j:32 * (j + 1), :], wev[:, :, j])
            wsel = sb.tile([P, 4, BL], FP32)
            for t in range(4):
                nc.vector.tensor_scalar_mul(wsel[:, t, :], s4, we128[:, t:t + 1])

            # final^T [d, b] = sum_{p,t} z[p, t, d] * wsel[p, t, b]
            pft = ps.tile([P, 2, BL], FP32, tag="gp", bufs=3)
            for dc in range(2):
                for t in range(4):
                    nc.tensor.matmul(
                        pft[:, dc, :],
                        z_sb[:, t, 128 * dc:128 * (dc + 1)],
                        wsel[:, t, :],
                        start=(t == 0), stop=(t == 3),
                    )
            ft16 = sb.tile([P, 2, BL], FP16)
            nc.vector.tensor_copy(ft16, pft)

            # logits [b, c] = sum_d final^T[d, b] * cq^T[d, c]   (fp16 mm)
            plog = ps.tile([BL, C], FP32, tag="gp", bufs=3)
            for dc in range(2):
                nc.tensor.matmul(
                    plog, ft16[:, dc, :], cqt[:, dc, :],
                    start=(dc == 0), stop=(dc == 1),
                )
            out_sb = sb.tile([BL, C], FP32)
            nc.vector.tensor_copy(out_sb, plog)
            nc.sync.dma_start(out, out_sb)

    nc.compile()
    # compile()'s move_matmul_waits_to_ldweights runs before the final ISA
    # lowering splits fused matmuls into Ldweights+Matmult, so a matmul can
    # still carry 2 waits (walrus MM struct fits only 1). Re-run the passes.
    import bass_rust
    bass_rust.move_matmul_waits_to_ldweights(nc.m)
    bass_rust.generate_event_semaphores(nc)
    for f in nc.m.functions:
        for blk in f.blocks:
            for inst in blk.instructions:
                w = inst.sync_info.on_wait if inst.sync_info else None
                if w and len(w) > 1 and "EventSemaphore" not in str(inst.opcode):
                    raise RuntimeError(
                        f"{inst.name} {inst.opcode} still has {len(w)} waits")
    return nc


_NC = None


def _get_nc():
    global _NC
    if _NC is None:
        _NC = _build_program()
    return _NC


def _host_consts(counts):
    # c32 [128, 64]: selp [:, :16], s4 [:, 16:48], crp rows 0:32 at [:, 48:64]
    c32 = np.zeros((P, 64), np.float32)
    for p in range(P):
        c32[p, (p % S) // L] = 1.0 / L          # selp (duplicated both halves)
        c32[p, E + p % BL] = 1.0                # s4 = tiled identity
    crp = np.log(counts.astype(np.float64) + 2.0).astype(np.float32)
    c32[0:BL, 48:48 + E] = np.broadcast_to(crp.reshape(1, E), (BL, E))
    return c32


def _make_in_maps(inputs):
    x = np.ascontiguousarray(np.asarray(inputs["x"], dtype=np.float32))
    queries = np.asarray(inputs["queries"], dtype=np.float64)
    Wk = np.asarray(inputs["Wk"], dtype=np.float64)
    Wv = np.asarray(inputs["Wv"], dtype=np.float32)
    cq = np.asarray(inputs["class_queries"], dtype=np.float32)
    counts = np.asarray(inputs["expert_counts"]).astype(np.float64)

    # qwT [128, 2, 64] fp16: (q @ Wk / 16)^T packed a-major
    qw = (np.einsum("eld,eda->ela", queries, Wk) / 16.0).astype(np.float32)
    qwT = qw.reshape(E * L, A).T.reshape(2, P, E * L).transpose(1, 0, 2)
    # cqT [128, 2, 100] fp16
    cqT = cq.T.reshape(2, P, C).transpose(1, 0, 2)
    c16 = np.concatenate(
        [qwT.reshape(P, 2 * S), cqT.reshape(P, 2 * C)], axis=1).astype(np.float16)
    c16 = np.ascontiguousarray(c16)
    c32 = _host_consts(counts)

    # wv [128, e, ac, d] fp32
    wvp = np.ascontiguousarray(
        Wv.transpose(0, 2, 1).reshape(E, 2, P, D).transpose(2, 0, 1, 3))

    in_maps = []
    for c in range(N_CORES):
        xl = x[BL * c:BL * (c + 1)].reshape(R, A)
        xTp = np.ascontiguousarray(
            xl.T.reshape(2, P, R).transpose(1, 0, 2)).astype(np.float16)
        xnp = np.ascontiguousarray(xl.reshape(R // P, P, A).transpose(1, 0, 2))
        in_maps.append({
            "xT": xTp,
            "xn": xnp,
            "wv": wvp,
            "c16": c16,
            "c32": c32,
        })
    return in_maps


def run_sharded(inputs, trace=False, **kwargs):
    nc = _get_nc()
    in_maps = _make_in_maps(inputs)
    res = run_bass_kernel_spmd(nc, in_maps, core_ids=list(range(N_CORES)),
                               trace=trace, **kwargs)
    outs = np.concatenate([res.results[c]["out"] for c in range(N_CORES)], axis=0)
    return outs.astype(np.float32), res


def kernel(**inputs):
    out, _ = run_sharded(inputs, trace=False)
    return out


# revision 7
# speedup vs baseline: 1.8158x; 1.3229x over previous
"""Trainium2 Bass kernel for CRPExpertAggregator (moe_routing).

Full-input contract: kernel(**inputs) takes the full unsharded inputs and
returns the full (256, 100) logits. Internally shards batch 8 ways across
NeuronCores 0-7 (data parallel; expert params replicated) and runs one SPMD
Bass program via concourse.bass_utils.run_bass_kernel_spmd.

Math (identical to the reference up to fp reassociation):
  H = x.reshape(B, 64, 256)
  scores[b,el,s] = sum_a (q@Wk/16)[el,a] * H[b,s,a]         (K never formed)
  attn = softmax_s(scores);  attn_avg[bs,e] = 0.25*sum_l attn
  U[b,e,a] = sum_s attn_avg * H;  z[b,e,d] = sum_a U * WvT  (V never formed)
  raw = ||z||, allsc = raw * log(counts+2), top-3 gate, logits = final @ cqT

Precision: the scores path streams fp16 (137x per-row safety on the top-3
gate). The U path needs fp32-level accuracy but fp32 stationaries cost a
213ns LDWEIGHTS per matmul, so attn/attn_avg/H stream as fp16 hi+lo PAIRS
(hi = fp16(x), lo = fp16(x - hi); products accumulate in fp32 PSUM; the
dropped lo*lo term is ~2^-22). The z path stays fully fp32 (rounding Wv or
U to single fp16 leaves <1.6x margin against the smallest rank-3/4 gap).

Perf structure (vs the 56.9us v1):
  - U/pav stationaries fp16 (1-pass LDWEIGHTS instead of 2-pass fp32);
    U streams the avt hi|lo pair concatenated so each stationary load
    serves two products (2 LW + 2 MM per chunk instead of 4 LW + 4 MM)
  - pft/plog run fp16 (z explicitly cast on PSUM evac; final rounding only)
  - raw = rawsq^0.5 on the DVE (AluOpType.pow) so the scalar engine keeps
    the Exp table loaded from scores to gate (saves 2x 1.28us table loads)
  - rw2 gate copies issue per-t inside the z section
"""

import numpy as np

import concourse.bass as bass
import concourse.bacc as bacc
import concourse.mybir as mybir
import concourse.tile as tile
from concourse.bass_utils import run_bass_kernel_spmd
from concourse.alu_op_type import AluOpType

FP32 = mybir.dt.float32
FP16 = mybir.dt.float16
AF = mybir.ActivationFunctionType
AX = mybir.AxisListType

N_CORES = 8
B = 256            # full batch
BL = B // N_CORES  # 32 rows per core
S = 64             # slots
A = 256            # agent dim (contraction for projections)
D = 256            # embed dim
E = 16             # experts
L = 4              # queries per expert
C = 100            # classes
R = BL * S         # 2048 H-rows per core
P = 128

C16W = 2 * S + 2 * C + E + BL  # qwt | cqt | selp | s4


def _build_program():
    nc = bacc.Bacc("TRN2", debug=False, enable_asserts=False, num_devices=N_CORES)

    # Host-packed DRAM inputs (exact SBUF layouts, partition dim first).
    xT = nc.dram_tensor("xT", (P, 2, R), FP16, kind="ExternalInput").ap()
    xnp = nc.dram_tensor("xnp", (P, R // P, 2, A), FP16, kind="ExternalInput").ap()
    wv = nc.dram_tensor("wv", (P, E, 2, D), FP32, kind="ExternalInput").ap()
    c16 = nc.dram_tensor("c16", (P, C16W), FP16, kind="ExternalInput").ap()
    c32 = nc.dram_tensor("c32", (P, E), FP32, kind="ExternalInput").ap()
    out = nc.dram_tensor("out", (BL, C), FP32, kind="ExternalOutput").ap()

    with tile.TileContext(nc) as tc:
        with tc.tile_pool(name="sb", bufs=1) as sb, \
             tc.tile_pool(name="ps", bufs=1, space="PSUM") as ps:
            # ---------------- DMA inputs (order = priority) ----------------
            c16_sb = sb.tile([P, C16W], FP16)
            nc.sync.dma_start(c16_sb, c16)
            xt_sb = sb.tile([P, 2, R], FP16)
            for ac in range(2):
                nc.sync.dma_start(xt_sb[:, ac], xT[:, ac])
            c32_sb = sb.tile([P, E], FP32)
            nc.sync.dma_start(c32_sb, c32)
            xn_sb = sb.tile([P, R // P, 2, A], FP16)  # [bs_p, rc, hi/lo, a]
            for h in range(2):
                nc.sync.dma_start(xn_sb[:, 8 * h:8 * (h + 1)], xnp[:, 8 * h:8 * (h + 1)])
            wv_sb = sb.tile([P, E, 2, D], FP32)
            for g in range(4):
                nc.sync.dma_start(wv_sb[:, 4 * g:4 * (g + 1)], wv[:, 4 * g:4 * (g + 1)])

            qwt = c16_sb[:, 0:2 * S].rearrange("p (ac el) -> p ac el", ac=2)
            cqt = c16_sb[:, 2 * S:2 * S + 2 * C].rearrange("p (dc c) -> p dc c", dc=2)
            selp = c16_sb[:, 2 * S + 2 * C:2 * S + 2 * C + E]
            s4 = c16_sb[:, 2 * S + 2 * C + E:]
            crp = c32_sb[0:BL, :]

            # Prefetch the Exp activation table while DMAs stream.
            warm_in = sb.tile([1, 1], FP32)
            warm_out = sb.tile([1, 2], FP32)
            nc.vector.memset(warm_in, 0.0)
            nc.scalar.activation(warm_out[:, 0:1], warm_in, AF.Exp)

            # ------- scores (fp16 mm, 2-way col tiling) -> exp -> normalize ----
            # attn layout [p = 64*h + el, bb = b%16, s]; h = b//16.
            attn = sb.tile([P, E, S], FP32)
            den = sb.tile([P, E], FP32)
            rden = sb.tile([P, E], FP32)
            ah = sb.tile([P, E, S], FP16)   # fp16 hi of normalized attn
            al = sb.tile([P, E, S], FP16)   # fp16 lo residual
            for it in range(2):
                psc = ps.tile([P, 8, S], FP32, tag="sc", bufs=2)
                for h in range(2):
                    for ac in range(2):
                        nc.tensor.matmul(
                            psc[64 * h:64 * (h + 1)].rearrange("p b s -> p (b s)"),
                            qwt[:, ac, :],
                            xt_sb[:, ac, 1024 * h + 512 * it:1024 * h + 512 * (it + 1)],
                            start=(ac == 0), stop=(ac == 1),
                            tile_position=(0, 64 * h),
                        )
                sl = slice(8 * it, 8 * (it + 1))
                nc.scalar.activation(attn[:, sl, :], psc, AF.Exp)
                nc.vector.reduce_sum(den[:, sl], attn[:, sl, :], axis=AX.X)
                nc.vector.reciprocal(rden[:, sl], den[:, sl])
                nc.vector.tensor_tensor(
                    attn[:, sl, :], attn[:, sl, :],
                    rden[:, sl, None].to_broadcast((P, 8, S)),
                    AluOpType.mult,
                )
                nc.scalar.copy(ah[:, sl, :], attn[:, sl, :])
                nc.vector.tensor_tensor(
                    al[:, sl, :], attn[:, sl, :], ah[:, sl, :], AluOpType.subtract)

            # ------- attn_avg^T (2-way row tiling, fp16 hi/lo stationaries) ----
            # avtp[r, rc, pair, par, e]: pair 0 = fp16 hi of attn_avg, pair 1 =
            # fp16 lo residual; parity par as before (complement rows zero).
            avtp = sb.tile([P, R // P, 2, 2, E], FP16)
            nc.vector.memset(avtp[S:P, :, :, 0, :], 0.0)
            nc.vector.memset(avtp[:S, :, :, 1, :], 0.0)
            pav0 = ps.tile([P, 8, E], FP32, tag="gp", bufs=3)
            pav1 = ps.tile([P, 8, E], FP32, tag="gp", bufs=3)
            pav = [pav0, pav1]
            for it in range(2):
                for k in range(4):
                    pl = 4 * it + k
                    for h in range(2):
                        for pr, src in ((0, ah), (1, al)):
                            nc.tensor.matmul(
                                pav[h][:, pl, :],
                                src[64 * h:64 * (h + 1), 2 * pl:2 * pl + 2, :]
                                .rearrange("p b s -> p (b s)"),
                                selp[64 * h:64 * (h + 1), :],
                                start=(pr == 0), stop=(pr == 1),
                                tile_position=(64 * h, 0),
                            )
            for h in range(2):
                hs = slice(8 * h, 8 * (h + 1))
                for par, rs in ((0, slice(0, S)), (1, slice(S, P))):
                    nc.vector.tensor_copy(avtp[rs, hs, 0, par, :], pav[h][rs])
                    nc.vector.tensor_tensor(
                        avtp[rs, hs, 1, par, :], pav[h][rs],
                        avtp[rs, hs, 0, par, :], AluOpType.subtract)

            # ------- U^T [a, b, e] = sum_s H^T attn_avg (fp16 pairs -> fp32) ---
            # Per chunk: Hh @ [avt_hi | avt_lo] (one LW, free=64), then
            # Hl @ avt_hi accumulating into the hi half (one LW, free=32).
            # ut = hi-half + lo-half of the psum afterwards.
            ut = sb.tile([P, 2, E, BL], FP32)  # [a_p, a_c, e, b]
            for ac in range(2):
                for half in range(2):
                    pu = ps.tile([P, 8, 2, 2, E], FP32, tag="gp", bufs=3)
                    for i in range(8):
                        rc = 8 * half + i
                        nc.tensor.matmul(
                            pu[:, i].rearrange("p pr par e -> p (pr par e)"),
                            xn_sb[:, rc, 0, 128 * ac:128 * (ac + 1)],
                            avtp[:, rc].rearrange("p pr par e -> p (pr par e)"),
                            start=True, stop=False,
                        )
                        nc.tensor.matmul(
                            pu[:, i, 0].rearrange("p par e -> p (par e)"),
                            xn_sb[:, rc, 1, 128 * ac:128 * (ac + 1)],
                            avtp[:, rc, 0].rearrange("p par e -> p (par e)"),
                            start=False, stop=True,
                            skip_group_check=True,
                        )
                    ut_view = ut[:, ac, :, 16 * half:16 * (half + 1)] \
                        .rearrange("p e (i par) -> p i par e", par=2)
                    nc.scalar.copy(ut_view, pu[:, :, 0])
                    nc.vector.tensor_tensor(ut_view, pu[:, :, 1], ut_view,
                                            AluOpType.add)

            # Prefetch the Sqrt table under the z section (dep on ut forces
            # this to execute after U, not at program start).
            nc.scalar.sqrt(warm_out[:, 1:2], ut[0:1, 0, 0, 0:1])

            # ------- z [32j+b, t, d], expert e = 4t+j (fp32, 4-way col tiling) -
            z_sb = sb.tile([P, 4, D], FP16)   # final-path copy (pft is fp16)
            zsq = sb.tile([P, 4, D], FP32)
            rawsq = sb.tile([P, 4], FP32)
            raw = sb.tile([P, 4], FP32)
            rw2 = sb.tile([BL, 4, 4], FP32)   # [b, t, j] -> free index e=4t+j
            for t in range(4):
                pz = ps.tile([P, D], FP32, tag="z", bufs=3)
                for j in range(4):
                    e = 4 * t + j
                    for ac in range(2):
                        nc.tensor.matmul(
                            pz[32 * j:32 * (j + 1), :],
                            ut[:, ac, e, :],
                            wv_sb[:, e, ac, :],
                            start=(ac == 0), stop=(ac == 1),
                            tile_position=(0, 32 * j),
                        )
                nc.scalar.activation(zsq[:, t, :], pz, AF.Square,
                                     accum_out=rawsq[:, t:t + 1])
                nc.vector.tensor_copy(z_sb[:, t, :], pz)

            # ---------------- allsc = raw * crp, top-3 gate --------------------
            nc.scalar.sqrt(raw, rawsq)
            for j in range(4):
                nc.vector.tensor_copy(rw2[:, :, j], raw[32 * j:32 * (j + 1), :])
            allsc = sb.tile([BL, E], FP32)
            nc.vector.tensor_tensor(
                allsc.rearrange("p (t j) -> p t j", j=4), rw2,
                crp.rearrange("p (t j) -> p t j", j=4), AluOpType.mult)
            mx8 = sb.tile([BL, 8], FP32)
            nc.vector.max(mx8, allsc)
            negm1 = sb.tile([BL, 1], FP32)
            nc.vector.tensor_scalar_mul(negm1, mx8[:, 0:1], -1.0)
            g = sb.tile([BL, E], FP32)
            nc.scalar.activation(g, allsc, AF.Exp, bias=negm1)
            gm = sb.tile([BL, E], FP32)
            nc.vector.scalar_tensor_tensor(
                gm, allsc, mx8[:, 2:3], g, AluOpType.is_ge, AluOpType.mult)
            ssum = sb.tile([BL, 1], FP32)
            nc.vector.reduce_sum(ssum, gm, axis=AX.X)
            rsum = sb.tile([BL, 1], FP32)
            nc.vector.reciprocal(rsum, ssum)
            we = sb.tile([BL, E], FP32)
            nc.vector.tensor_scalar_mul(we, gm, rsum)

            # scatter we [b, e] -> we128 [32j+b, t]; wsel[p, t, b] = s4 * we128
            we128 = sb.tile([P, 4], FP32)
            wev = we.rearrange("p (t j) -> p t j", j=4)
            for j in range(4):
                nc.vector.tensor_copy(we128[32 * j:32 * (j + 1), :], wev[:, :, j])
            wsel = sb.tile([P, 4, BL], FP16)
            for t in range(4):
                nc.vector.tensor_scalar_mul(wsel[:, t, :], s4, we128[:, t:t + 1])

            # final^T [d, b] = sum_{p,t} z[p, t, d] * wsel[p, t, b]   (fp16 mm)
            pft = ps.tile([P, 2, BL], FP32, tag="gp", bufs=3)
            for dc in range(2):
                for t in range(4):
                    nc.tensor.matmul(
                        pft[:, dc, :],
                        z_sb[:, t, 128 * dc:128 * (dc + 1)],
                        wsel[:, t, :],
                        start=(t == 0), stop=(t == 3),
                    )
            ft16 = sb.tile([P, 2, BL], FP16)
            nc.vector.tensor_copy(ft16, pft)

            # logits [b, c] = sum_d final^T[d, b] * cq^T[d, c]   (fp16 mm)
            plog = ps.tile([BL, C], FP32, tag="gp", bufs=3)
            for dc in range(2):
                nc.tensor.matmul(
                    plog, ft16[:, dc, :], cqt[:, dc, :],
                    start=(dc == 0), stop=(dc == 1),
                )
            out_sb = sb.tile([BL, C], FP32)
            nc.vector.tensor_copy(out_sb, plog)
            nc.sync.dma_start(out, out_sb)

    nc.compile()
    # compile()'s move_matmul_waits_to_ldweights runs before the final ISA
    # lowering splits fused matmuls into Ldweights+Matmult, so a matmul can
    # still carry 2 waits (walrus MM struct fits only 1). Re-run the passes.
    import bass_rust
    bass_rust.move_matmul_waits_to_ldweights(nc.m)
    bass_rust.generate_event_semaphores(nc)
    for f in nc.m.functions:
        for blk in f.blocks:
            for inst in blk.instructions:
                w = inst.sync_info.on_wait if inst.sync_info else None
                if w and len(w) > 1 and "EventSemaphore" not in str(inst.opcode):
                    raise RuntimeError(
                        f"{inst.name} {inst.opcode} still has {len(w)} waits")
    return nc


_NC = None


def _get_nc():
    global _NC
    if _NC is None:
        _NC = _build_program()
    return _NC


def _make_in_maps(inputs):
    x = np.ascontiguousarray(np.asarray(inputs["x"], dtype=np.float32))
    queries = np.asarray(inputs["queries"], dtype=np.float64)
    Wk = np.asarray(inputs["Wk"], dtype=np.float64)
    Wv = np.asarray(inputs["Wv"], dtype=np.float32)
    cq = np.asarray(inputs["class_queries"], dtype=np.float32)
    counts = np.asarray(inputs["expert_counts"]).astype(np.float64)

    # c16 [128, C16W] fp16: qwt | cqt | selp | s4
    qw = (np.einsum("eld,eda->ela", queries, Wk) / 16.0).astype(np.float32)
    qwT = qw.reshape(E * L, A).T.reshape(2, P, E * L).transpose(1, 0, 2)
    cqT = cq.T.reshape(2, P, C).transpose(1, 0, 2)
    selp = np.zeros((P, E), np.float32)
    s4 = np.zeros((P, BL), np.float32)
    for p in range(P):
        selp[p, (p % S) // L] = 0.25
        s4[p, p % BL] = 1.0
    c16 = np.concatenate(
        [qwT.reshape(P, 2 * S), cqT.reshape(P, 2 * C), selp, s4],
        axis=1).astype(np.float16)
    c16 = np.ascontiguousarray(c16)

    c32 = np.zeros((P, E), np.float32)
    c32[0:BL, :] = np.log(counts + 2.0).astype(np.float32).reshape(1, E)

    # wv [128, e, ac, d] fp32
    wvp = np.ascontiguousarray(
        Wv.transpose(0, 2, 1).reshape(E, 2, P, D).transpose(2, 0, 1, 3))

    in_maps = []
    for c in range(N_CORES):
        xl = x[BL * c:BL * (c + 1)].reshape(R, A)
        xTp = np.ascontiguousarray(
            xl.T.reshape(2, P, R).transpose(1, 0, 2)).astype(np.float16)
        # xnp [128, rc, hi/lo, a] fp16 pair of the natural layout
        xn = xl.reshape(R // P, P, A).transpose(1, 0, 2)
        hi = xn.astype(np.float16)
        lo = (xn - hi.astype(np.float32)).astype(np.float16)
        xnpair = np.ascontiguousarray(np.stack([hi, lo], axis=2))
        in_maps.append({
            "xT": xTp,
            "xnp": xnpair,
            "wv": wvp,
            "c16": c16,
            "c32": c32,
        })
    return in_maps


def run_sharded(inputs, trace=False, **kwargs):
    nc = _get_nc()
    in_maps = _make_in_maps(inputs)
    res = run_bass_kernel_spmd(nc, in_maps, core_ids=list(range(N_CORES)),
                               trace=trace, **kwargs)
    outs = np.concatenate([res.results[c]["out"] for c in range(N_CORES)], axis=0)
    return outs.astype(np.float32), res


def kernel(**inputs):
    out, _ = run_sharded(inputs, trace=False)
    return out
